# revision 17
# baseline (speedup 1.0000x reference)
"""Trainium2 Bass kernel for nn_CapsuleLayer_4372276707524.

Math (per row r=(b,u,n,c), vector over d of size D=16):
  p_d = w[u,n,c,d] * v[b,c,u]          (pondered)
  3 routing iterations of:
    c = softmax(l); out = squash(c*p); l += p*out
  returns out of the last iteration, laid out [b, n, u, c, d].

Closed-form chain (exact): with E = sum_d e, S = sum_d (e*p)^2 the squash+
softmax normalizer collapses to  alpha = sqrt(S) / (E^2 + S)  (eps -> 0),
and the exp shifts cancel termwise:
  it1: e=1 -> beta1 = sqrt(s1)/(256+s1), s1 = a^2 * W2 (host W2 = sum_d w^2)
  it2: x2 = beta1*a^2*w^2; e2 = exp(x2-12); abar2 = sqrt(S2)/(E2^2+S2)
  it3: l3 = x2*(1 + (abar2/beta1)*e2); e3 = exp(l3-20); out = abar3*(e3*p)
sqrt via exp(0.5*ln(.)) so the ACT engine stays on one table set.

dtypes: w shipped bf16 (p-path) AND as f32 w^2 (exp-arg path); e/u/out bf16;
all exp arguments and [P,80] chain scalars f32.

Sharding: data-parallel over batch, 4 batches per core across 8 cores.
"""

import sys

import numpy as np
import ml_dtypes

if "/opt/trn_rl_repo" not in sys.path:
    sys.path.insert(0, "/opt/trn_rl_repo")

import concourse.bass as bass
import concourse.tile as tile
from concourse import bacc, mybir
from concourse.bass import AP
from concourse.bass_utils import run_bass_kernel_spmd

F32 = mybir.dt.float32
BF16 = mybir.dt.bfloat16
BF = ml_dtypes.bfloat16
AF = mybir.ActivationFunctionType
OP = mybir.AluOpType
SH2 = 12.0
SH3 = 20.0

B_FULL = 32
N_CORES = 8
B_CORE = B_FULL // N_CORES  # 4
U = 1152
N = 10
C = 8
D = 16
UC = 9  # u chunks of 128
P = 128
CD = C * D  # 128
NC_ = N * C  # 80
NCD = N * C * D  # 1280

# ---------------------------------------------------------------------------
# Activation-table monkeypatch: route Exp/Ln/Square to the ONE table set that
# contains all three (natural_log_exp_and_others) -> single ACT_TABLE_LOAD.
_TABLES_PATCHED = False


def _patch_act_tables():
    global _TABLES_PATCHED
    if _TABLES_PATCHED:
        return
    from concourse import hw_specs
    orig = hw_specs.get_activation_tables
    combo = {AF.Exp, AF.Ln, AF.Square}
    target = "natural_log_exp_and_others"

    def patched(arch):
        tabs = orig(arch)
        out = {}
        for name, funcs in tabs.items():
            if name == target:
                out[name] = set(funcs)
            else:
                out[name] = {f for f in funcs if f not in combo}
        return out

    hw_specs.get_activation_tables = patched
    import concourse.bacc as bacc_mod
    if hasattr(bacc_mod, "get_activation_tables"):
        bacc_mod.get_activation_tables = patched
    _TABLES_PATCHED = True


def _bc(ap: AP, axis: int, n: int) -> AP:
    """Insert a broadcast (stride 0) dim at free-axis position `axis`."""
    dims = [list(x) for x in ap.ap]
    dims.insert(axis + 1, [0, n])
    return AP(ap.tensor, ap.offset, dims)


def build_program(n_uc=UC, n_b=B_CORE):
    _patch_act_tables()
    nc = bacc.Bacc(
        "TRN2",
        target_bir_lowering=False,
        debug=False,
        num_devices=1,
    )
    wb_d = nc.dram_tensor("wb", (n_uc, P, NCD), BF16, kind="ExternalInput").ap()
    wsq_d = nc.dram_tensor("wsq", (n_uc, P, NCD), BF16, kind="ExternalInput").ap()
    w2s_d = nc.dram_tensor("w2s", (n_uc, P, NC_), F32, kind="ExternalInput").ap()
    vb_d = nc.dram_tensor("vb", (n_b, n_uc, P, CD), BF16, kind="ExternalInput").ap()
    a2_d = nc.dram_tensor("a2", (n_b, n_uc, P, C), F32, kind="ExternalInput").ap()
    out_d = nc.dram_tensor(
        "out", (n_b, N, n_uc, P, CD), BF16, kind="ExternalOutput"
    ).ap()
    emit(nc, wb_d, wsq_d, w2s_d, vb_d, a2_d, out_d, n_uc, n_b)
    nc.compile()
    return nc


def emit(nc, wb_d, wsq_d, w2s_d, vb_d, a2_d, out_d, n_uc, n_b):
    with tile.TileContext(nc) as tc:
        with (
            tc.tile_pool(name="const", bufs=1) as cpool,
            tc.tile_pool(name="wres", bufs=2) as wpool,
            tc.tile_pool(name="vin", bufs=4) as vpool,
            tc.tile_pool(name="bigf", bufs=3) as fpool,
            tc.tile_pool(name="bigb", bufs=3) as bpool,
            tc.tile_pool(name="small", bufs=4) as spool,
            tc.tile_pool(name="tree", bufs=3) as tpool,
            tc.tile_pool(name="outp", bufs=3) as opool,
        ):
            b2_t = cpool.tile([P, 1], F32, tag="b2c")
            nc.vector.memset(b2_t[:], -SH2)
            b3_t = cpool.tile([P, 1], F32, tag="b3c")
            nc.vector.memset(b3_t[:], -SH3)

            def treesum(big, R, tag):
                """R[P,80] f32 = sum_d big[P,(80,16)] (plain reduce; the
                halving-add variant inflates under SBUF contention)."""
                nc.vector.reduce_sum(
                    R[:], big[:].rearrange("p (k d) -> p k d", d=D),
                    axis=mybir.AxisListType.X)

            def chain(E, S, alpha, adtype_note=None):
                """alpha = sqrt(S)/(E^2+S); alpha tile provided (bf16 or f32).
                sqrt via exp(0.5*ln), all on the exp/ln/square table."""
                lnS = spool.tile([P, NC_], F32, tag="c_ln")
                nc.scalar.activation(lnS[:], S[:], AF.Ln)
                sS = spool.tile([P, NC_], F32, tag="c_s")
                nc.scalar.activation(sS[:], lnS[:], AF.Exp, scale=0.5)
                Esq = spool.tile([P, NC_], F32, tag="c_esq")
                nc.scalar.activation(Esq[:], E[:], AF.Square)
                Dt = spool.tile([P, NC_], F32, tag="c_d")
                nc.vector.tensor_add(Dt[:], Esq[:], S[:])
                rD = spool.tile([P, NC_], F32, tag="c_rd")
                nc.vector.reciprocal_approx_fast(rD[:], Dt[:])
                nc.gpsimd.tensor_mul(alpha[:], sS[:], rD[:])

            w_sb = {}

            def get_w(uc):
                if uc not in w_sb:
                    wb_sb = wpool.tile([P, NCD], BF16, tag="wb")
                    nc.sync.dma_start(wb_sb[:], wb_d[uc])
                    wsq_sb = wpool.tile([P, NCD], BF16, tag="wsq")
                    nc.sync.dma_start(wsq_sb[:], wsq_d[uc])
                    w2s_sb = wpool.tile([P, NC_], F32, tag="w2s")
                    nc.sync.dma_start(w2s_sb[:], w2s_d[uc])
                    w_sb[uc] = (wb_sb, wsq_sb, w2s_sb)
                return w_sb[uc]

            def tile_stages(uc, b):
                st = {}

                def s0():
                    wb_sb, wsq_sb, w2s_sb = get_w(uc)
                    st.update(wb=wb_sb, wsq=wsq_sb, w2s=w2s_sb)
                    vb = vpool.tile([P, CD], BF16, tag="vb")
                    nc.sync.dma_start(vb[:], vb_d[b, uc])
                    a2t = vpool.tile([P, C], F32, tag="a2t")
                    nc.sync.dma_start(a2t[:], a2_d[b, uc])
                    # s1 = w2s * a2 (bcast over n)
                    s1 = spool.tile([P, NC_], F32, tag="s1")
                    nc.gpsimd.tensor_mul(
                        s1[:].rearrange("p (n c) -> p n c", n=N),
                        w2s_sb[:].rearrange("p (n c) -> p n c", n=N),
                        _bc(a2t[:], 0, N))
                    # noqa: s1 name shadows stage fn list below intentionally
                    # beta1 = sqrt(s1)/(256+s1)
                    lns = spool.tile([P, NC_], F32, tag="b_ln")
                    nc.scalar.activation(lns[:], s1[:], AF.Ln)
                    ss = spool.tile([P, NC_], F32, tag="b_s")
                    nc.scalar.activation(ss[:], lns[:], AF.Exp, scale=0.5)
                    D1 = spool.tile([P, NC_], F32, tag="b_d")
                    nc.vector.tensor_scalar_add(D1[:], s1[:], 256.0)
                    rD1 = spool.tile([P, NC_], F32, tag="b_rd")
                    nc.vector.reciprocal_approx_fast(rD1[:], D1[:])
                    beta1 = spool.tile([P, NC_], F32, tag="beta1")
                    nc.gpsimd.tensor_mul(beta1[:], ss[:], rD1[:])
                    rb1 = spool.tile([P, NC_], F32, tag="rb1")
                    nc.vector.reciprocal_approx_fast(rb1[:], beta1[:])
                    # ba = beta1 * a2 (bcast over n) -> bf16
                    ba = spool.tile([P, NC_], BF16, tag="ba")
                    nc.gpsimd.tensor_mul(
                        ba[:].rearrange("p (n c) -> p n c", n=N),
                        beta1[:].rearrange("p (n c) -> p n c", n=N),
                        _bc(a2t[:], 0, N))
                    st.update(vb=vb, ba=ba, rb1=rb1)

                def s1():
                    # x2 = ba (bcast d) * wsq   [bf16]
                    x2 = fpool.tile([P, NCD], BF16, tag="x2")
                    nc.gpsimd.tensor_mul(
                        x2[:].rearrange("p (k d) -> p k d", d=D),
                        st["wsq"][:].rearrange("p (k d) -> p k d", d=D),
                        _bc(st["ba"][:], 1, D))
                    # p = vb (bcast n) * wb    [bf16]
                    p = bpool.tile([P, NCD], BF16, tag="p")
                    nc.vector.tensor_mul(
                        p[:].rearrange("p (n k) -> p n k", n=N),
                        st["wb"][:].rearrange("p (n k) -> p n k", n=N),
                        _bc(st["vb"][:], 0, N))
                    st.update(x2=x2, p=p)

                def s2():
                    e2 = bpool.tile([P, NCD], BF16, tag="e2")
                    nc.scalar.activation(e2[:], st["x2"][:], AF.Exp,
                                         bias=b2_t[:])
                    st.update(e2=e2)

                def s3():
                    E2 = spool.tile([P, NC_], F32, tag="E2")
                    treesum(st["e2"], E2, "tr_e2")
                    u2 = bpool.tile([P, NCD], BF16, tag="u2")
                    nc.vector.tensor_mul(u2[:], st["e2"][:], st["p"][:])
                    st.update(E2=E2, u2=u2)

                def s4():
                    usq2 = bpool.tile([P, NCD], BF16, tag="usq2")
                    nc.scalar.activation(usq2[:], st["u2"][:], AF.Square)
                    S2 = spool.tile([P, NC_], F32, tag="S2")
                    treesum(usq2, S2, "tr_s2")
                    abar2 = spool.tile([P, NC_], F32, tag="abar2")
                    chain(st["E2"], S2, abar2)
                    g2 = spool.tile([P, NC_], BF16, tag="g2")
                    nc.gpsimd.tensor_mul(g2[:], abar2[:], st["rb1"][:])
                    st.update(g2=g2)

                def s5():
                    # t = g2 (bcast d) * e2   [bf16]
                    t = bpool.tile([P, NCD], BF16, tag="t")
                    nc.gpsimd.tensor_mul(
                        t[:].rearrange("p (k d) -> p k d", d=D),
                        st["e2"][:].rearrange("p (k d) -> p k d", d=D),
                        _bc(st["g2"][:], 1, D))
                    # l3 = (t + 1) * x2       [bf16]
                    l3 = fpool.tile([P, NCD], BF16, tag="l3")
                    nc.vector.scalar_tensor_tensor(
                        l3[:], t[:], 1.0, st["x2"][:], OP.add, OP.mult)
                    st.update(l3=l3)

                def s6():
                    e3 = bpool.tile([P, NCD], BF16, tag="e3")
                    nc.scalar.activation(e3[:], st["l3"][:], AF.Exp,
                                         bias=b3_t[:])
                    st.update(e3=e3)

                def s7():
                    E3 = spool.tile([P, NC_], F32, tag="E3")
                    treesum(st["e3"], E3, "tr_e3")
                    u3 = bpool.tile([P, NCD], BF16, tag="u3")
                    nc.vector.tensor_mul(u3[:], st["e3"][:], st["p"][:])
                    st.update(E3=E3, u3=u3)

                def s8():
                    usq3 = bpool.tile([P, NCD], BF16, tag="usq3")
                    nc.scalar.activation(usq3[:], st["u3"][:], AF.Square)
                    S3 = spool.tile([P, NC_], F32, tag="S3")
                    treesum(usq3, S3, "tr_s3")
                    abar3 = spool.tile([P, NC_], BF16, tag="abar3")
                    chain(st["E3"], S3, abar3)
                    st.update(abar3=abar3)

                def s9():
                    outt = opool.tile([P, NCD], BF16, tag="outt")
                    nc.gpsimd.tensor_mul(
                        outt[:].rearrange("p (k d) -> p k d", d=D),
                        st["u3"][:].rearrange("p (k d) -> p k d", d=D),
                        _bc(st["abar3"][:], 1, D))
                    dst = out_d[b, :, uc].rearrange("n p cd -> p n cd")
                    nc.sync.dma_start(
                        dst, outt[:].rearrange("p (n cd) -> p n cd", n=N))

                return [s0, s1, s2, s3, s4, s5, s6, s7, s8, s9]

            # staggered pipeline: DELTA stages between consecutive tiles,
            # so ceil(NSTAGE/DELTA) tiles are in flight at once.
            NSTAGE = 10
            DELTA = 4
            fns = [tile_stages(uc, b)
                   for uc in range(n_uc) for b in range(n_b)]
            T = len(fns)
            for k in range((T - 1) * DELTA + NSTAGE):
                for i in range(T):
                    s = k - i * DELTA
                    if 0 <= s < NSTAGE:
                        fns[i][s]()


def _host_prep(inputs: np.ndarray, weights: np.ndarray, n_uc=UC):
    wf = weights.reshape(U, NCD)
    wb = np.ascontiguousarray(wf.reshape(n_uc, P, NCD)).astype(BF)
    wsq = np.ascontiguousarray(
        (wf.astype(np.float32) ** 2).reshape(n_uc, P, NCD)).astype(BF)
    w2 = (weights.astype(np.float32) ** 2).sum(axis=-1)  # [U,N,C]
    w2s = np.ascontiguousarray(w2.reshape(n_uc, P, NC_)).astype(np.float32)
    vt = np.ascontiguousarray(inputs.transpose(0, 2, 1))  # [B, U, C]
    vbb = np.broadcast_to(vt[:, :, :, None], (B_FULL, U, C, D))
    vb = np.ascontiguousarray(vbb).reshape(B_FULL, n_uc, P, CD).astype(BF)
    a2 = np.ascontiguousarray(
        (vt.astype(np.float32) ** 2).reshape(B_FULL, n_uc, P, C))
    return wb, wsq, w2s, vb, a2


_NC_CACHE = {}


def _get_program():
    key = "full"
    if key not in _NC_CACHE:
        _NC_CACHE[key] = build_program()
    return _NC_CACHE[key]


def kernel(inputs: np.ndarray, weights: np.ndarray, _trace=False) -> np.ndarray:
    inputs = np.asarray(inputs, dtype=np.float32)
    weights = np.asarray(weights, dtype=np.float32)
    assert inputs.shape == (B_FULL, C, U), inputs.shape
    assert weights.shape == (U, N, C, D), weights.shape

    wb, wsq, w2s, vb, a2 = _host_prep(inputs, weights)
    nc = _get_program()
    in_maps = []
    for core in range(N_CORES):
        bs = slice(core * B_CORE, (core + 1) * B_CORE)
        in_maps.append({
            "wb": wb,
            "wsq": wsq,
            "w2s": w2s,
            "vb": np.ascontiguousarray(vb[bs]),
            "a2": np.ascontiguousarray(a2[bs]),
        })
    res = run_bass_kernel_spmd(
        nc, in_maps, list(range(N_CORES)), trace=_trace)
    outs = []
    for core in range(N_CORES):
        o = np.asarray(res.results[core]["out"])  # [B_CORE, N, UC, P, CD] bf16
        outs.append(o.reshape(B_CORE, N, UC * P, C, D))
    full = np.concatenate(outs, axis=0).astype(np.float32)
    if _trace:
        kernel.last_exec_time_ns = res.exec_time_ns
    return full


kernel.last_exec_time_ns = None


if __name__ == "__main__":
    rng = np.random.default_rng(0)
    inputs = rng.standard_normal((B_FULL, C, U), dtype=np.float32)
    weights = rng.standard_normal((U, N, C, D), dtype=np.float32)
    out = kernel(inputs, weights)
    print("out shape", out.shape, out.dtype)
